# revision 3
# baseline (speedup 1.0000x reference)
"""Trainium2 Bass kernel for Coo2FulSimple (periodic pairwise squared
distances + cutoff adjacency mask).

Contract: kernel(**inputs) takes the FULL unsharded inputs (numpy) and
returns the FULL outputs (out [B,N,N,S] f32, mask [B,N,N,S] bool).

Sharding: 16 units = (batch b, i-tile of 128 atoms) distributed 2 per
core across 8 NeuronCores. Each core computes its [2,128,512,27] slab.

Algorithm (minimum-image structure): for a cubic 20A box with rc=6,
per axis c at most ONE of the three shift values k in {-1,0,1} can give
a squared displacement W_c[k] = fl(fl(d_c + t_k)^2) <= rc^2 (the
candidate intervals |d|<=6, d>=14, d<=-14 are disjoint, and IEEE
round-to-nearest keeps fl(a+b) >= max(a,b) for a,b >= 0, so
sod <= rc^2 implies every axis term <= rc^2).  Hence the [27] output
row of each (i,j) pair is one-hot: the only candidate is
s* = (k0*,k1*,k2*) with k_c* = argmin_k W_c[k], and its value is
msum = fl(fl(m0+m1)+m2) with m_c = min_k W_c[k] -- bit-identical to
the reference sod at s*.  The dense 27-wide tensors are then outer
products of 3-wide indicators:

  ind_c[k] = (W_c[k] <= rc^2)                     exact f32 compare
  out27 = (ind0*fl16(msum)) x ind1 x ind2         fp16, one rounding
  mask27 = (out27 != 0)                           u8; self pairs give
                                                  msum=+0.0 -> mask=0.

out is returned as fp16 widened to f32 on the host (single fp16
rounding, rel err <= 2^-11 ~ 4.9e-4, inside the 2e-2 gate); the mask
is exact (all comparisons on exact-f32 values).

Engine split per 256-column j-chunk (balanced ~7.5us each):
  Pool: D = pos_i - pos_j; msum tree tail; indicators; G; T0 (f16)
  ACT : W = Square(D + t_k) (fused bias-add+square); mask rows 0:MA
        via Sign(out27)
  DVE : per-axis min over shifts; 9/27-wide fp16 outer products
        (2x packed mode); mask rows MA:27 via (out27 != 0)
  DMA : out27 fp16 per chunk (contiguous 13.8KB/partition), mask u8
        per unit (contiguous 4.6-11KB/partition)

Pool (GpSimd) ISA gaps found empirically: TensorTensor min and
scalar_tensor_tensor are rejected by walrus on Pool; tensor_tensor
add/mult and tensor(-single)-scalar is_le are fine.
"""

import os
from contextlib import ExitStack

import numpy as np

B, N, S = 4, 512, 27
NCORES = 8
IT = 128          # i-tile size == SBUF partitions
JC = 256          # j-chunk size
NCH = N // JC     # j-chunks per unit
UNITS = 2         # units per core
RC2 = 36.0
MA = 23           # mask rows on ACT (Sign); rows MA:27 on DVE

CW = 3 * N + 3 * UNITS + 3   # [ negpj (3*512) | posi (3*UNITS) | tau (3) ]
P0 = 3 * N
TV0 = 3 * N + 3 * UNITS

_CACHE = {}


def _build_program():
    import concourse.bacc as bacc
    import concourse.mybir as mybir
    import concourse.tile as tile

    f32 = mybir.dt.float32
    f16 = mybir.dt.float16
    u8 = mybir.dt.uint8
    SQUARE = mybir.ActivationFunctionType.Square
    SIGN = mybir.ActivationFunctionType.Sign
    ADD = mybir.AluOpType.add
    MULT = mybir.AluOpType.mult
    MIN = mybir.AluOpType.min
    IS_LE = mybir.AluOpType.is_le
    NE = mybir.AluOpType.not_equal

    nc = bacc.Bacc(
        "TRN2", target_bir_lowering=False, debug=False, num_devices=NCORES
    )

    cst = nc.dram_tensor("cst", [IT, CW], f32, kind="ExternalInput").ap()
    outv = nc.dram_tensor(
        "outv", [UNITS, IT, NCH, S, JC], f16, kind="ExternalOutput"
    ).ap()
    outm = nc.dram_tensor("outm", [UNITS, IT, S, N], u8, kind="ExternalOutput").ap()

    # Walrus codegen supports very few embedded semaphore waits per compute
    # instruction: ops are arranged to carry at most ONE cross-engine RAW
    # wait, and cross-engine WAR hazards on rotated pool buffers are absorbed
    # by 1-element "carrier" ops on the writing engine just before the real
    # producer.  Scratch rows are grouped into one tile per (engine, dtype)
    # so each tile has a single cross-engine reader set.
    #
    # Row maps:  PF32 [6,JC] f32 (Pool): D 0:3 | s1 3 | msum 4 | gi 5
    #            DV32 [6,JC] f32 (DVE):  mA 0:3 | m 3:6
    #            PF16 [13,JC] f16 (Pool): ind0 0:3 | ind1 3:6 | ind2 6:9 |
    #                                     T0 9:12 | G 12
    with ExitStack() as ctx:
        tc = ctx.enter_context(tile.TileContext(nc))
        const = ctx.enter_context(tc.tile_pool(name="const", bufs=1))
        cst_sb = const.tile([IT, CW], f32)
        nc.sync.dma_start(cst_sb[:], cst)
        npj = cst_sb[:, 0 : 3 * N].rearrange("p (c j) -> p c j", j=N)

        pf32p = ctx.enter_context(tc.tile_pool(name="pf32", bufs=2))
        wpool = ctx.enter_context(tc.tile_pool(name="wpool", bufs=2))
        dv32p = ctx.enter_context(tc.tile_pool(name="dv32", bufs=2))
        pf16p = ctx.enter_context(tc.tile_pool(name="pf16", bufs=2))
        v9pool = ctx.enter_context(tc.tile_pool(name="v9pool", bufs=2))
        opool = ctx.enter_context(tc.tile_pool(name="opool", bufs=2))
        mkpool = ctx.enter_context(tc.tile_pool(name="mkpool", bufs=2))

        for u in range(UNITS):
            mk = mkpool.tile([IT, S, N], u8)
            # mk WAR carriers (absorb previous generation's mask DMA reads)
            nc.scalar.mul(mk[0:1, 0:1, 0:1], mk[0:1, 0:1, 0:1], 0.0)
            nc.vector.memset(mk[0:1, MA : MA + 1, 0:1], 0)

            for h in range(NCH):
                j0 = h * JC

                # --- Pool: D_c = (-pos_j) + pos_i
                PF32 = pf32p.tile([IT, 6, JC], f32)
                nc.gpsimd.memset(PF32[0:1, 0:1, 0:1], 0.0)  # WAR (ACT reads D)
                posi = (
                    cst_sb[:, P0 + 3 * u : P0 + 3 * u + 3]
                    .unsqueeze(2)
                    .broadcast_to([IT, 3, JC])
                )
                D = PF32[:, 0:3, :]
                nc.gpsimd.tensor_tensor(D, npj[:, :, j0 : j0 + JC], posi, ADD)

                # --- ACT: W[3k+c] = Square(D_c + tau_k), k-grouped rows
                W9 = wpool.tile([IT, 9, JC], f32)
                # WAR carriers: Pool (inds) first, then DVE (mins)
                nc.scalar.mul(W9[0:1, 0:1, 0:1], W9[0:1, 0:1, 0:1], 0.0)
                nc.scalar.mul(W9[0:1, 8:9, 0:1], W9[0:1, 8:9, 0:1], 0.0)
                for k in range(3):
                    nc.scalar.activation(
                        W9[:, 3 * k : 3 * k + 3, :],
                        D,
                        SQUARE,
                        bias=cst_sb[:, TV0 + k : TV0 + k + 1],
                        scale=1.0,
                    )
                Wk = W9[:].rearrange("p (k c) j -> p k c j", c=3)

                # --- DVE: per-axis minimum over the three shifts
                DV32 = dv32p.tile([IT, 6, JC], f32)
                nc.vector.memset(DV32[0:1, 3:4, 0:1], 0)  # WAR (Pool reads m)
                mA = DV32[:, 0:3, :]
                m = DV32[:, 3:6, :]
                nc.vector.tensor_tensor(mA, W9[:, 0:3, :], W9[:, 3:6, :], MIN)
                nc.vector.tensor_tensor(m, mA, W9[:, 6:9, :], MIN)

                # --- Pool: msum = (m0 + m1) + m2, gate gi, G = fl16 value
                s1 = PF32[:, 3:4, :]
                msum = PF32[:, 4:5, :]
                gi = PF32[:, 5:6, :]
                nc.gpsimd.tensor_tensor(s1, m[:, 0:1, :], m[:, 1:2, :], ADD)
                nc.gpsimd.tensor_tensor(msum, s1, m[:, 2:3, :], ADD)
                nc.gpsimd.tensor_single_scalar(gi, msum, RC2, IS_LE)

                PF16 = pf16p.tile([IT, 13, JC], f16)
                nc.gpsimd.memset(PF16[0:1, 0:1, 0:1], 0)  # WAR (DVE readers)
                G = PF16[:, 12:13, :]
                nc.gpsimd.tensor_tensor(G, gi, msum, MULT)
                # indicators ind_c[k] = (W_c[k] <= rc^2) as f16 rows
                for c in range(3):
                    nc.gpsimd.tensor_single_scalar(
                        PF16[:, 3 * c : 3 * c + 3, :], Wk[:, :, c, :], RC2, IS_LE
                    )
                T0 = PF16[:, 9:12, :]
                nc.gpsimd.tensor_tensor(
                    T0, PF16[:, 0:3, :], G.broadcast_to([IT, 3, JC]), MULT
                )

                # --- DVE: V9 = T0 x ind1;  out27 = V9 x ind2  (fp16, 2x)
                V9 = v9pool.tile([IT, 9, JC], f16)
                nc.vector.tensor_tensor(
                    V9[:].rearrange("p (a b) j -> p a b j", b=3),
                    T0.unsqueeze(2).broadcast_to([IT, 3, 3, JC]),
                    PF16[:, 3:6, :].unsqueeze(1).broadcast_to([IT, 3, 3, JC]),
                    MULT,
                )
                o27 = opool.tile([IT, S, JC], f16)
                nc.vector.memset(o27[0:1, S - 1 : S, 0:1], 0)  # WAR (DMA)
                nc.vector.memset(o27[0:1, 0:1, 0:1], 0)  # WAR (ACT)
                nc.vector.tensor_tensor(
                    o27[:].rearrange("p (a b) j -> p a b j", b=3),
                    V9[:].unsqueeze(2).broadcast_to([IT, 9, 3, JC]),
                    PF16[:, 6:9, :].unsqueeze(1).broadcast_to([IT, 9, 3, JC]),
                    MULT,
                )

                # --- mask bytes: ACT rows 0:MA via Sign, DVE rows MA:S
                nc.scalar.activation(
                    mk[:, 0:MA, j0 : j0 + JC], o27[:, 0:MA, :], SIGN
                )
                nc.vector.tensor_single_scalar(
                    mk[:, MA:S, j0 : j0 + JC], o27[:, MA:S, :], 0.0, NE
                )

                # --- DMA: out chunk (13.8KB contiguous per partition)
                nc.sync.dma_start(outv[u, :, h, :, :], o27[:])

            # --- DMA: mask for the unit (contiguous per partition)
            nc.sync.dma_start(outm[u, :, 0:MA, :], mk[:, 0:MA, :])
            nc.sync.dma_start(outm[u, :, MA:S, :], mk[:, MA:S, :])

    nc.compile()
    return nc


def _get_program():
    if "nc" not in _CACHE:
        _CACHE["nc"] = _build_program()
    return _CACHE["nc"]


def _prep_core_inputs(pos, tau):
    """Per-core input dicts. Core k: batch k//2, i-tiles 2*(k%2), 2*(k%2)+1."""
    in_maps = []
    for k in range(NCORES):
        b = k // 2
        it0 = 2 * (k % 2)
        cst = np.empty((IT, CW), np.float32)
        cst[:, : 3 * N] = (-pos[b].T).reshape(1, 3 * N)
        for u in range(UNITS):
            i0 = (it0 + u) * IT
            cst[:, P0 + 3 * u : P0 + 3 * u + 3] = pos[b, i0 : i0 + IT, :]
        cst[:, TV0 : TV0 + 3] = tau.reshape(1, 3)
        in_maps.append({"cst": cst})
    return in_maps


def _gather(results):
    out = np.empty((B, N, N, S), np.float32)
    mask = np.empty((B, N, N, S), np.uint8)
    for k in range(NCORES):
        b = k // 2
        it0 = 2 * (k % 2)
        ov = results[k]["outv"]  # [UNITS, IT, NCH, S, JC] f16
        om = results[k]["outm"]  # [UNITS, IT, S, N] u8
        for u in range(UNITS):
            i0 = (it0 + u) * IT
            for h in range(NCH):
                out[b, i0 : i0 + IT, h * JC : (h + 1) * JC, :] = (
                    ov[u][:, h].transpose(0, 2, 1).astype(np.float32)
                )
            mask[b, i0 : i0 + IT] = om[u].transpose(0, 2, 1)
    return out, mask


def _analyze_shifts(cel_mat, sft_cel):
    """Return tau[3] f32 if inputs have the standard structure (uniform
    diagonal cell, sft = meshgrid(-1..1)^3 so s = 9*k0 + 3*k1 + k2), else
    None.  tau[k] = fl((k-1) * L) is the exact Cartesian shift value per
    axis (identical across axes for the cubic cell)."""
    r = np.arange(-1, 2)
    expect = np.stack(np.meshgrid(r, r, r, indexing="ij"), axis=-1).reshape(-1, 3)
    if sft_cel.shape != (27, 3) or not np.array_equal(sft_cel, expect):
        return None
    cel0 = cel_mat[0]
    if not np.all(cel_mat == cel0[None]):
        return None
    if np.any(cel0 != np.diag(np.diag(cel0))):
        return None
    diag = np.diag(cel0).astype(np.float32)
    if not (diag[0] == diag[1] == diag[2]):
        return None
    tau = np.empty(3, np.float32)
    for k in range(3):
        tau[k] = np.float32(np.float32(k - 1) * diag[0])
    return tau


def _reference_fallback(pos_xyz, cel_mat, pbc, ent, sft_cel):
    """Plain numpy mirror of the reference (for non-standard inputs only)."""
    sft_xyz = np.einsum("sd,bde->bse", sft_cel.astype(cel_mat.dtype), cel_mat)
    vec = (
        pos_xyz[:, :, None, None, :]
        - pos_xyz[:, None, :, None, :]
        + sft_xyz[:, None, None, :, :]
    )
    sod = np.sum(vec * vec, axis=-1)
    n = pos_xyz.shape[1]
    eye = np.eye(n, dtype=bool)
    zero_sft = np.all(sft_cel == 0, axis=-1)
    self_pair = eye[None, :, :, None] & zero_sft[None, None, None, :]
    val = ent[:, :, None, None] & ent[:, None, :, None]
    mask = (sod <= RC2) & val & ~self_pair
    out = np.where(mask, sod, np.zeros((), sod.dtype))
    return out, mask


def kernel(pos_xyz, cel_mat, pbc, ent, sft_cel):
    pos_xyz = np.asarray(pos_xyz)
    cel_mat = np.asarray(cel_mat)
    pbc = np.asarray(pbc)
    ent = np.asarray(ent)
    sft_cel = np.asarray(sft_cel)

    tau = None
    if pos_xyz.shape == (B, N, 3) and pos_xyz.dtype == np.float32:
        tau = _analyze_shifts(cel_mat, sft_cel)
    if tau is None:
        return _reference_fallback(pos_xyz, cel_mat, pbc, ent, sft_cel)

    from concourse.bass_utils import run_bass_kernel_spmd

    nc = _get_program()
    in_maps = _prep_core_inputs(pos_xyz, tau)
    trace = os.environ.get("BENCH_TRACE", "") == "1"
    res = run_bass_kernel_spmd(
        nc, in_maps, core_ids=list(range(NCORES)), trace=trace
    )
    _CACHE["last_results"] = res
    out, mask = _gather(res.results)

    # Self pairs come out as out=0/mask=0 on device (msum == +0.0 there),
    # matching the reference exactly; only entity masking needs host glue
    # for generality (ent is all-True for the standard inputs).
    if not ent.all():
        val = ent[:, :, None, None] & ent[:, None, :, None]
        mask &= val[..., None].astype(np.uint8)
        out *= mask
    return out, mask.view(np.bool_)


# revision 7
# speedup vs baseline: 2.0056x; 2.0056x over previous
"""Trainium2 Bass kernel for Coo2FulSimple (periodic pairwise squared
distances + cutoff adjacency mask).

Contract: kernel(**inputs) takes the FULL unsharded inputs (numpy) and
returns the FULL outputs (out [B,N,N,S] f32, mask [B,N,N,S] bool).

Sharding: 16 units = (batch b, i-tile of 128 atoms) distributed 2 per
core across 8 NeuronCores. Each core computes its [2,128,512,27] slab.

Algorithm (minimum-image structure): for a cubic 20A box with rc=6,
per axis c at most ONE of the three shift values k in {-1,0,1} can give
a squared displacement W_c[k] = fl(fl(d_c + t_k)^2) <= rc^2 (the
candidate intervals |d|<=6, d>=14, d<=-14 are disjoint, and IEEE
round-to-nearest keeps fl(a+b) >= max(a,b) for a,b >= 0, so
sod <= rc^2 implies every axis term <= rc^2).  Hence the [27] output
row of each (i,j) pair is one-hot: the only candidate is
s* = (k0*,k1*,k2*) with k_c* = argmin_k W_c[k], and its value is
msum = fl(fl(m0+m1)+m2) with m_c = min_k W_c[k] -- bit-identical to
the reference sod at s*.  The dense 27-wide tensors are then outer
products of 3-wide indicators:

  ind_c[k] = (W_c[k] <= rc^2)                     exact f32 compare
  out27 = (ind0*fl16(msum)) x ind1 x ind2         fp16, one rounding
  mask27 = (out27 != 0)                           u8; self pairs give
                                                  msum=+0.0 -> mask=0.

out is returned as fp16 widened to f32 on the host (single fp16
rounding, rel err <= 2^-11 ~ 4.9e-4, inside the 2e-2 gate); the mask
is exact (all comparisons on exact-f32 values).

Engine split per 256-column j-chunk (balanced ~7.5us each):
  Pool: D = pos_i - pos_j; msum tree tail; indicators; G; T0 (f16)
  ACT : W = Square(D + t_k) (fused bias-add+square); mask rows 0:MA
        via Sign(out27)
  DVE : per-axis min over shifts; 9/27-wide fp16 outer products
        (2x packed mode); mask rows MA:27 via (out27 != 0)
  DMA : out27 fp16 per chunk (contiguous 13.8KB/partition), mask u8
        per unit (contiguous 4.6-11KB/partition)

Pool (GpSimd) ISA gaps found empirically: TensorTensor min and
scalar_tensor_tensor are rejected by walrus on Pool; tensor_tensor
add/mult and tensor(-single)-scalar is_le are fine.
"""

import os
from contextlib import ExitStack

import numpy as np

B, N, S = 4, 512, 27
NCORES = 8
IT = 128          # i-tile size == SBUF partitions
JC = 256          # j-chunk size
NCH = N // JC     # j-chunks per unit
UNITS = 2         # units per core
RC2 = 36.0
MA = 23           # mask rows on ACT (Sign); rows MA:27 on DVE

CW = 3 * N + 3 * UNITS + 3   # [ negpj (3*512) | posi (3*UNITS) | tau (3) ]
P0 = 3 * N
TV0 = 3 * N + 3 * UNITS

_CACHE = {}


def _build_program():
    import concourse.bacc as bacc
    import concourse.mybir as mybir
    import concourse.tile as tile

    f32 = mybir.dt.float32
    f16 = mybir.dt.float16
    u8 = mybir.dt.uint8
    SQUARE = mybir.ActivationFunctionType.Square
    SIGN = mybir.ActivationFunctionType.Sign
    ADD = mybir.AluOpType.add
    MULT = mybir.AluOpType.mult
    MIN = mybir.AluOpType.min
    IS_LE = mybir.AluOpType.is_le
    NE = mybir.AluOpType.not_equal

    nc = bacc.Bacc(
        "TRN2", target_bir_lowering=False, debug=False, num_devices=NCORES
    )

    cst = nc.dram_tensor("cst", [IT, CW], f32, kind="ExternalInput").ap()
    outv = nc.dram_tensor(
        "outv", [UNITS, IT, NCH, S, JC], f16, kind="ExternalOutput"
    ).ap()
    outm = nc.dram_tensor("outm", [UNITS, IT, S, N], u8, kind="ExternalOutput").ap()

    # Walrus codegen supports very few embedded semaphore waits per compute
    # instruction: ops are arranged to carry at most ONE cross-engine RAW
    # wait, and cross-engine WAR hazards on rotated pool buffers are absorbed
    # by 1-element "carrier" ops on the writing engine just before the real
    # producer.  Scratch rows are grouped into one tile per (engine, dtype)
    # so each tile has a single cross-engine reader set.
    #
    # Row maps:  PF32 [6,JC] f32 (Pool): D 0:3 | s1 3 | msum 4 | gi 5
    #            DV32 [6,JC] f32 (DVE):  mA 0:3 | m 3:6
    #            PF16 [13,JC] f16 (Pool): ind0 0:3 | ind1 3:6 | ind2 6:9 |
    #                                     T0 9:12 | G 12
    with ExitStack() as ctx:
        tc = ctx.enter_context(tile.TileContext(nc))
        const = ctx.enter_context(tc.tile_pool(name="const", bufs=1))
        cst_sb = const.tile([IT, CW], f32)
        nc.sync.dma_start(cst_sb[:], cst)
        npj = cst_sb[:, 0 : 3 * N].rearrange("p (c j) -> p c j", j=N)

        pf32p = ctx.enter_context(tc.tile_pool(name="pf32", bufs=2))
        wpool = ctx.enter_context(tc.tile_pool(name="wpool", bufs=2))
        dv32p = ctx.enter_context(tc.tile_pool(name="dv32", bufs=2))
        pf16p = ctx.enter_context(tc.tile_pool(name="pf16", bufs=2))
        v9pool = ctx.enter_context(tc.tile_pool(name="v9pool", bufs=2))
        opool = ctx.enter_context(tc.tile_pool(name="opool", bufs=2))
        mkpool = ctx.enter_context(tc.tile_pool(name="mkpool", bufs=2))
        scrp = ctx.enter_context(tc.tile_pool(name="scrp", bufs=1))
        scr = scrp.tile([IT, 1], f32)

        for u in range(UNITS):
            mk = mkpool.tile([IT, S, N], u8)
            # mk WAR carrier (absorb previous generation's mask DMA read)
            nc.scalar.mul(mk[0:1, 0:1, 0:1], mk[0:1, 0:1, 0:1], 0.0)

            for h in range(NCH):
                j0 = h * JC

                # --- Pool: D_c = (-pos_j) + pos_i
                PF32 = pf32p.tile([IT, 3, JC], f32)
                nc.gpsimd.memset(PF32[0:1, 0:1, 0:1], 0.0)  # WAR (ACT reads D)
                posi = (
                    cst_sb[:, P0 + 3 * u : P0 + 3 * u + 3]
                    .unsqueeze(2)
                    .broadcast_to([IT, 3, JC])
                )
                D = PF32[:, 0:3, :]
                nc.gpsimd.tensor_tensor(D, npj[:, :, j0 : j0 + JC], posi, ADD)

                # --- ACT: W[3k+c] = Square(D_c + tau_k), k-grouped rows
                W9 = wpool.tile([IT, 9, JC], f32)
                nc.scalar.mul(W9[0:1, 0:1, 0:1], W9[0:1, 0:1, 0:1], 0.0)  # WAR (DVE)
                for k in range(3):
                    nc.scalar.activation(
                        W9[:, 3 * k : 3 * k + 3, :],
                        D,
                        SQUARE,
                        bias=cst_sb[:, TV0 + k : TV0 + k + 1],
                        scale=1.0,
                    )
                Wk = W9[:].rearrange("p (k c) j -> p k c j", c=3)

                # --- DVE: per-axis minimum, msum = (m0+m1)+m2, gate gi
                DV32 = dv32p.tile([IT, 9, JC], f32)
                nc.vector.memset(DV32[0:1, 6:7, 0:1], 0)  # WAR (Pool reads)
                mA = DV32[:, 0:3, :]
                m = DV32[:, 3:6, :]
                s1 = DV32[:, 6:7, :]
                msum = DV32[:, 7:8, :]
                gi = DV32[:, 8:9, :]
                nc.vector.tensor_tensor(mA, W9[:, 0:3, :], W9[:, 3:6, :], MIN)
                nc.vector.tensor_tensor(m, mA, W9[:, 6:9, :], MIN)
                nc.vector.tensor_tensor(s1, m[:, 0:1, :], m[:, 1:2, :], ADD)
                nc.vector.tensor_tensor(msum, s1, m[:, 2:3, :], ADD)
                nc.vector.tensor_single_scalar(gi, msum, RC2, IS_LE)

                # --- DVE: indicators ind_c[k] = (W_c[k] <= rc^2) as f16
                # PF16 rows: ind0 0:3 | ind1 3:6 | ind2 6:9 | T0 9:12 | G 12
                PF16 = pf16p.tile([IT, 13, JC], f16)
                nc.vector.memset(PF16[0:1, 0:1, 0:1], 0)  # WAR (Pool readers)
                nc.gpsimd.memset(PF16[0:1, 12:13, 0:1], 0)  # WAR (DVE readers)
                for c in range(3):
                    nc.vector.tensor_single_scalar(
                        PF16[:, 3 * c : 3 * c + 3, :], Wk[:, :, c, :], RC2, IS_LE
                    )

                # --- Pool: G = gi * msum (fp16), T0 = ind0 * G
                G = PF16[:, 12:13, :]
                nc.gpsimd.tensor_tensor(G, gi, msum, MULT)
                T0 = PF16[:, 9:12, :]
                nc.gpsimd.tensor_tensor(
                    T0, PF16[:, 0:3, :], G.broadcast_to([IT, 3, JC]), MULT
                )

                # --- V9 = T0 x ind1 (DVE); out27 = V9 x ind2 in k0-slabs:
                # rows 0:9 on DVE, rows 9:27 on Pool.
                V9 = v9pool.tile([IT, 9, JC], f16)
                nc.vector.tensor_tensor(
                    V9[:].rearrange("p (a b) j -> p a b j", b=3),
                    T0.unsqueeze(2).broadcast_to([IT, 3, 3, JC]),
                    PF16[:, 3:6, :].unsqueeze(1).broadcast_to([IT, 3, 3, JC]),
                    MULT,
                )
                o27 = opool.tile([IT, S, JC], f16)
                nc.vector.memset(o27[0:1, 8:9, 0:1], 0)  # WAR (DMA)
                nc.vector.memset(o27[0:1, 0:1, 0:1], 0)  # WAR (ACT)
                nc.gpsimd.memset(o27[0:1, S - 1 : S, 0:1], 0)  # WAR (DMA)
                nc.gpsimd.memset(o27[0:1, 9:10, 0:1], 0)  # WAR (ACT)
                ind2 = PF16[:, 6:9, :]
                o27v = o27[:].rearrange("p (a b) j -> p a b j", b=3)
                nc.vector.tensor_tensor(
                    o27v[:, 0:3, :, :],
                    V9[:, 0:3, :].unsqueeze(2).broadcast_to([IT, 3, 3, JC]),
                    ind2.unsqueeze(1).broadcast_to([IT, 3, 3, JC]),
                    MULT,
                )
                nc.gpsimd.tensor_tensor(
                    o27v[:, 3:9, :, :],
                    V9[:, 3:9, :].unsqueeze(2).broadcast_to([IT, 6, 3, JC]),
                    ind2.unsqueeze(1).broadcast_to([IT, 6, 3, JC]),
                    MULT,
                )

                # --- ACT: mask bytes via Sign (rows split by o27 writer)
                nc.scalar.activation(
                    mk[:, 0:9, j0 : j0 + JC], o27[:, 0:9, :], SIGN
                )
                nc.scalar.activation(
                    mk[:, 9:S, j0 : j0 + JC], o27[:, 9:S, :], SIGN
                )

                # --- DMA: out chunk (13.8KB contiguous per partition).
                # Funnel: a 1-elem Pool read of the DVE-written slab lets the
                # DMA wait on Pool alone (covers both o27 writers).
                nc.gpsimd.tensor_scalar_add(scr[:, 0:1], o27[:, 0, 0:1], 0.0)
                nc.sync.dma_start(outv[u, :, h, :, :], o27[:])

            # --- DMA: mask for the unit (contiguous per partition)
            nc.sync.dma_start(outm[u, :, :, :], mk[:])

    nc.compile()
    return nc


def _get_program():
    if "nc" not in _CACHE:
        _CACHE["nc"] = _build_program()
    return _CACHE["nc"]


def _prep_core_inputs(pos, tau):
    """Per-core input dicts. Core k: batch k//2, i-tiles 2*(k%2), 2*(k%2)+1."""
    in_maps = []
    for k in range(NCORES):
        b = k // 2
        it0 = 2 * (k % 2)
        cst = np.empty((IT, CW), np.float32)
        cst[:, : 3 * N] = (-pos[b].T).reshape(1, 3 * N)
        for u in range(UNITS):
            i0 = (it0 + u) * IT
            cst[:, P0 + 3 * u : P0 + 3 * u + 3] = pos[b, i0 : i0 + IT, :]
        cst[:, TV0 : TV0 + 3] = tau.reshape(1, 3)
        in_maps.append({"cst": cst})
    return in_maps


def _gather(results):
    out = np.empty((B, N, N, S), np.float32)
    mask = np.empty((B, N, N, S), np.uint8)
    for k in range(NCORES):
        b = k // 2
        it0 = 2 * (k % 2)
        ov = results[k]["outv"]  # [UNITS, IT, NCH, S, JC] f16
        om = results[k]["outm"]  # [UNITS, IT, S, N] u8
        for u in range(UNITS):
            i0 = (it0 + u) * IT
            for h in range(NCH):
                out[b, i0 : i0 + IT, h * JC : (h + 1) * JC, :] = (
                    ov[u][:, h].transpose(0, 2, 1).astype(np.float32)
                )
            mask[b, i0 : i0 + IT] = om[u].transpose(0, 2, 1)
    return out, mask


def _analyze_shifts(cel_mat, sft_cel):
    """Return tau[3] f32 if inputs have the standard structure (uniform
    diagonal cell, sft = meshgrid(-1..1)^3 so s = 9*k0 + 3*k1 + k2), else
    None.  tau[k] = fl((k-1) * L) is the exact Cartesian shift value per
    axis (identical across axes for the cubic cell)."""
    r = np.arange(-1, 2)
    expect = np.stack(np.meshgrid(r, r, r, indexing="ij"), axis=-1).reshape(-1, 3)
    if sft_cel.shape != (27, 3) or not np.array_equal(sft_cel, expect):
        return None
    cel0 = cel_mat[0]
    if not np.all(cel_mat == cel0[None]):
        return None
    if np.any(cel0 != np.diag(np.diag(cel0))):
        return None
    diag = np.diag(cel0).astype(np.float32)
    if not (diag[0] == diag[1] == diag[2]):
        return None
    tau = np.empty(3, np.float32)
    for k in range(3):
        tau[k] = np.float32(np.float32(k - 1) * diag[0])
    return tau


def _reference_fallback(pos_xyz, cel_mat, pbc, ent, sft_cel):
    """Plain numpy mirror of the reference (for non-standard inputs only)."""
    sft_xyz = np.einsum("sd,bde->bse", sft_cel.astype(cel_mat.dtype), cel_mat)
    vec = (
        pos_xyz[:, :, None, None, :]
        - pos_xyz[:, None, :, None, :]
        + sft_xyz[:, None, None, :, :]
    )
    sod = np.sum(vec * vec, axis=-1)
    n = pos_xyz.shape[1]
    eye = np.eye(n, dtype=bool)
    zero_sft = np.all(sft_cel == 0, axis=-1)
    self_pair = eye[None, :, :, None] & zero_sft[None, None, None, :]
    val = ent[:, :, None, None] & ent[:, None, :, None]
    mask = (sod <= RC2) & val & ~self_pair
    out = np.where(mask, sod, np.zeros((), sod.dtype))
    return out, mask


def kernel(pos_xyz, cel_mat, pbc, ent, sft_cel):
    pos_xyz = np.asarray(pos_xyz)
    cel_mat = np.asarray(cel_mat)
    pbc = np.asarray(pbc)
    ent = np.asarray(ent)
    sft_cel = np.asarray(sft_cel)

    tau = None
    if pos_xyz.shape == (B, N, 3) and pos_xyz.dtype == np.float32:
        tau = _analyze_shifts(cel_mat, sft_cel)
    if tau is None:
        return _reference_fallback(pos_xyz, cel_mat, pbc, ent, sft_cel)

    from concourse.bass_utils import run_bass_kernel_spmd

    nc = _get_program()
    in_maps = _prep_core_inputs(pos_xyz, tau)
    trace = os.environ.get("BENCH_TRACE", "") == "1"
    res = run_bass_kernel_spmd(
        nc, in_maps, core_ids=list(range(NCORES)), trace=trace
    )
    _CACHE["last_results"] = res
    out, mask = _gather(res.results)

    # Self pairs come out as out=0/mask=0 on device (msum == +0.0 there),
    # matching the reference exactly; only entity masking needs host glue
    # for generality (ent is all-True for the standard inputs).
    if not ent.all():
        val = ent[:, :, None, None] & ent[:, None, :, None]
        mask &= val[..., None].astype(np.uint8)
        out *= mask
    return out, mask.view(np.bool_)
